# revision 29
# baseline (speedup 1.0000x reference)
"""DCM (dynamic conv module) Trainium2 kernel, v3.

Reference computation (per sample b, channel c):
  f[b,c,3,3]  = adaptive_avg_pool2d(x[b,c], 3)        # dynamic depthwise filter
  out[b,c]    = depthwise_conv3x3(x[b,c], f[b,c])     # zero padding 1
  y           = relu(batchnorm_train(out, gamma, beta))  # batch stats over (B,H,W)

Sharding: data-parallel over batch B=16 across 8 cores (2 samples/core).
Sync-BN via a [C,2] AllReduce of per-channel (sum, sumsq).

v3 design:
  - x shipped fp16 (host cast), resident in SBUF once per plane in a PACKED
    layout [C, 1 + 130*128 + 1]: one zero row above and below the 128 x rows
    plus a single zero element at each end. Rows are contiguous (stride W),
    so every tap window is a FLAT [C, 2048] slice: fast contiguous DMA in,
    fast PE moving operands, clean 2D DVE/ACT access patterns.
  - Horizontal zero-padding: flat dj=+-1 windows wrap across row ends; the
    wrapped contributions are removed by per-plane correction columns
    (corr0/corr127 [C,128], 3 small stt each) subtracted from the two edge
    columns of each result tile (2 small strided stt per tile).
  - The 9 taps are spread across engines (measured rates: PE ~0.77ns/col,
    ACT ~1.2GHz, DVE ~1cyc/elem, no DVE fast mode for accumulating ops):
      ACT 1 tap  (0,0): activation Copy w/ per-channel scale writes PSUM
      PE  5 taps: fp16 diag matmuls accumulate on top (start=False)
      DVE 3 taps: stt merge psum->res (fp16), then 2 stt RMW on res;
                  the last one emits the per-channel sum via accum_out
  - Pooling on ACT as 9 rectangle sums per plane (Copy + accum_out);
    sumsq on ACT (Square + accum_out). Same activation table -> no reloads.
  - BN apply on DVE/ACT in fp16, output DMA'd fp16; host casts to f32.
"""

import numpy as np

# ---------------------------------------------------------------- constants
B, C, H, W = 16, 128, 128, 128
N_CORES = 8
BL = B // N_CORES          # samples per core
HW = H * W
FS = 3
BN_EPS = 1e-5

XF = 1 + (H + 3) * W       # packed plane: 1 front pad, zero row, 128 x rows,
                           # zero row, then a zero row of slack for views
XOFF = 1 + W               # flat offset of x row 0 inside the plane tile

TROWS = 16                 # output rows per psum tile
NT = H // TROWS            # tiles per plane (8)
TF = TROWS * W             # 2048 free elems per psum tile
NTILE = NT * BL            # psum tiles per core (16)
MM_N = 512                 # max moving free per matmul (psum bank limit)

# adaptive_avg_pool2d(3) bin boundaries (PyTorch convention)
SH = [(i * H) // FS for i in range(FS)]
EH = [-((-(i + 1) * H) // FS) for i in range(FS)]
SW = [(i * W) // FS for i in range(FS)]
EW = [-((-(i + 1) * W) // FS) for i in range(FS)]

# taps t = 3*(di+1)+(dj+1), (di,dj) in {-1,0,1}^2
TAPS = [(di, dj) for di in (-1, 0, 1) for dj in (-1, 0, 1)]
T_ACT = 4                  # (0,0)  ACT tap (SBUF fp16, folded by GPSIMD)
T_PE = [0, 1, 2, 6]        # (-1,*) and (1,-1): PE diag matmuls
T_SHARED = 7               # (1,0)  PE on even tiles, DVE stt on odd tiles
T_MERGE = 5                # (0,1)  DVE stt: res = x*f + psum (fp16 out)
T_RMW = 3                  # (0,-1) DVE stt RMW on res
T_LAST = 8                 # (1,1)  DVE stt RMW on res, accum_out -> sums
DJM = [0, 3, 6]            # taps with dj=-1 (wrap at col 0)
DJP = [2, 5, 8]            # taps with dj=+1 (wrap at col W-1)


def _counts_recip():
    cr = np.empty((C, FS * FS), dtype=np.float32)
    for i in range(FS):
        for j in range(FS):
            cr[:, 3 * i + j] = 1.0 / float((EH[i] - SH[i]) * (EW[j] - SW[j]))
    return cr


def build_nc(n_cores: int = N_CORES):
    """Build + compile the per-core Bass program (identical on all cores)."""
    import concourse.bacc as bacc
    import concourse.tile as tile
    from concourse import mybir

    f32 = mybir.dt.float32
    f16 = mybir.dt.float16
    AT = mybir.ActivationFunctionType
    OP = mybir.AluOpType
    AX = mybir.AxisListType

    ntot = float(n_cores * BL * HW)   # BN element count per channel

    nc = bacc.Bacc(
        "TRN2",
        target_bir_lowering=False,
        debug=False,
        num_devices=n_cores,
    )

    x_d = nc.dram_tensor("x", [BL, C, HW], f16, kind="ExternalInput").ap()
    gamma_d = nc.dram_tensor("gamma", [C, 1], f32, kind="ExternalInput").ap()
    beta_d = nc.dram_tensor("beta", [C, 1], f32, kind="ExternalInput").ap()
    ident_d = nc.dram_tensor("ident", [C, C], f16, kind="ExternalInput").ap()
    crecip_d = nc.dram_tensor("crecip", [C, FS * FS], f32, kind="ExternalInput").ap()
    y_d = nc.dram_tensor("y", [BL, C, HW], f16, kind="ExternalOutput").ap()

    with tile.TileContext(nc) as tc:
        with (
            tc.tile_pool(name="singles", bufs=1) as singles,
            tc.tile_pool(name="xres", bufs=BL) as xresp,
            tc.tile_pool(name="res", bufs=BL) as resp,
            tc.tile_pool(name="psum", bufs=2, space="PSUM") as psum,
            tc.tile_pool(name="scr", bufs=2) as scrp,
            tc.tile_pool(name="sq", bufs=2) as sqp,
            tc.tile_pool(name="fpool", bufs=2) as fpool,
            tc.tile_pool(name="diagp", bufs=2 * len(T_PE)) as diagp,
            tc.tile_pool(name="corrp", bufs=2 * 2) as corrp,
            tc.tile_pool(name="statp", bufs=1) as statp,
            tc.tile_pool(name="dram", bufs=1, space="DRAM") as dram,
        ):
            # ---- constants
            gamma_s = singles.tile([C, 1], f32, tag="gamma")
            nc.sync.dma_start(out=gamma_s[:], in_=gamma_d[:, :])
            beta_s = singles.tile([C, 1], f32, tag="beta")
            nc.sync.dma_start(out=beta_s[:], in_=beta_d[:, :])
            ident_s = singles.tile([C, C], f16, tag="ident")
            nc.sync.dma_start(out=ident_s[:], in_=ident_d[:, :])
            crecip_s = singles.tile([C, FS * FS], f32, tag="crecip")
            nc.sync.dma_start(out=crecip_s[:], in_=crecip_d[:, :])

            sums = statp.tile([C, NTILE], f32, tag="sums")
            sumsq = statp.tile([C, NTILE], f32, tag="sumsq")

            # Dummy warm-up AllReduce issued at kernel start: absorbs the
            # one-time ncfw ramp so the real stats AllReduce is cheaper.
            warm = statp.tile([C, 2], f32, tag="warm")
            nc.gpsimd.memset(warm[:], 0.0)
            dw_in = dram.tile([C, 2], f32, tag="dw_in")
            dw_out = dram.tile([C, 2], f32, tag="dw_out")
            nc.sync.dma_start(out=dw_in[:], in_=warm[:])
            for _ in range(2):
                nc.gpsimd.collective_compute(
                    "AllReduce",
                    OP.add,
                    replica_groups=[list(range(n_cores))],
                    ins=[dw_in[:].opt()],
                    outs=[dw_out[:].opt()],
                )

            # ---- resident x planes (packed, padded rows) + result planes
            xts, rts = [], []
            for s in range(BL):
                xt = xresp.tile([C, XF], f16, tag="xres")
                nc.vector.memset(xt[:, 0:XOFF], 0.0)            # front pad + row -1
                nc.vector.memset(xt[:, XOFF + HW:XF], 0.0)      # rows 128..129+
                xts.append(xt)
                rt = resp.tile([C, HW], f16, tag="res")
                rts.append(rt)

            # chunked contiguous plane loads; issue across three engine
            # queues in parallel (each dma_start costs ~0.7us of issue time
            # on its queue, which serializes the transfers if queued on one)
            NLCH = 8
            LCF = HW // NLCH
            for s in range(BL):
                for ci in range(NLCH):
                    nc.sync.dma_start(
                        out=xts[s][:, XOFF + ci * LCF:XOFF + (ci + 1) * LCF],
                        in_=x_d[s, :, ci * LCF:(ci + 1) * LCF],
                    )

            fTs, diags, corr0s, corr127s, rects_ = {}, {}, {}, {}, {}

            def plane_prep_act(s):
                """Pool rectangle sums on ACT (emit early)."""
                xt = xts[s]
                xv = xt[:, XOFF:XOFF + HW].rearrange("p (r w) -> p r w", w=W)
                rect = fpool.tile([C, FS * FS], f32, tag="rect")
                scr = scrp.tile([C, 44 * 44], f16, tag="scr")
                for i in range(FS):
                    bh = EH[i] - SH[i]
                    for j in range(FS):
                        bw = EW[j] - SW[j]
                        sv = scr[:, 0:bh * bw].rearrange("p (r w) -> p r w", w=bw)
                        nc.scalar.activation(
                            out=sv,
                            in_=xv[:, SH[i]:EH[i], SW[j]:EW[j]],
                            func=AT.Copy,
                            accum_out=rect[:, 3 * i + j:3 * i + j + 1],
                        )
                rects_[s] = rect

            def plane_prep_dve(s):
                """f, diag weights, wrap-correction cols (DVE, emit late
                enough that the pool rects have landed)."""
                xt = xts[s]
                fT = fpool.tile([C, FS * FS], f32, tag="fT")
                nc.vector.tensor_mul(fT[:], rects_[s][:], crecip_s[:])
                fTs[s] = fT
                dgs = {}
                for t in T_PE + [T_SHARED]:
                    dg = diagp.tile([C, C], f16, tag="diag")
                    nc.vector.tensor_scalar_mul(dg[:], ident_s[:], fT[:, t:t + 1])
                    dgs[t] = dg
                diags[s] = dgs
                # wrap-correction columns:
                # corr0[r]   = sum_di f(di,-1) * xpad[row r+di,   col 127]
                # corr127[r] = sum_di f(di,+1) * xpad[row r+di+2, col 0]
                c0 = corrp.tile([C, H], f32, tag="corr0")
                c1 = corrp.tile([C, H], f32, tag="corr127")
                for n, t in enumerate(DJM):
                    di = TAPS[t][0]
                    a = (di + 1) * W
                    v = xt[:, a:a + HW].rearrange(
                        "p (r w) -> p r w", w=W
                    )[:, :, 0:1]
                    if n == 0:
                        nc.vector.tensor_scalar_mul(
                            c0[:].rearrange("p (r w) -> p r w", w=1),
                            v, fT[:, t:t + 1],
                        )
                    else:
                        nc.vector.scalar_tensor_tensor(
                            out=c0[:].rearrange("p (r w) -> p r w", w=1),
                            in0=v, scalar=fT[:, t:t + 1],
                            in1=c0[:].rearrange("p (r w) -> p r w", w=1),
                            op0=OP.mult, op1=OP.add,
                        )
                for n, t in enumerate(DJP):
                    di = TAPS[t][0]
                    a = 1 + (di + 2) * W
                    v = xt[:, a:a + HW].rearrange(
                        "p (r w) -> p r w", w=W
                    )[:, :, 0:1]
                    if n == 0:
                        nc.vector.tensor_scalar_mul(
                            c1[:].rearrange("p (r w) -> p r w", w=1),
                            v, fT[:, t:t + 1],
                        )
                    else:
                        nc.vector.scalar_tensor_tensor(
                            out=c1[:].rearrange("p (r w) -> p r w", w=1),
                            in0=v, scalar=fT[:, t:t + 1],
                            in1=c1[:].rearrange("p (r w) -> p r w", w=1),
                            op0=OP.mult, op1=OP.add,
                        )
                corr0s[s], corr127s[s] = c0, c1

            def tapwin(s, r0, t, n=TF):
                """Flat [C, n] window of x for tap t covering out rows r0.."""
                di, dj = TAPS[t]
                ofs = 1 + (r0 + di + 1) * W + dj
                return xts[s][:, ofs:ofs + n]

            def conv_tile(k):
                s, i = divmod(k, NT)
                r0 = i * TROWS
                fT = fTs[s]
                pt = psum.tile([C, TF], f32, tag="pt")
                # PE diag-matmul taps accumulate in psum; tap T_SHARED
                # alternates PE/DVE by tile parity to balance the engines
                pe_taps = list(T_PE) + ([T_SHARED] if k % 2 == 0 else [])
                for n, t in enumerate(pe_taps):
                    last = n == len(pe_taps) - 1
                    for h in range(TF // MM_N):
                        nc.tensor.matmul(
                            pt[:, h * MM_N:(h + 1) * MM_N],
                            diags[s][t][:],
                            tapwin(s, r0, t)[:, h * MM_N:(h + 1) * MM_N],
                            start=n == 0,
                            stop=last,
                        )
                # ACT computes tap T_ACT into an SBUF fp16 tile (folded in
                # below by a GPSIMD tensor add)
                a0 = sqp.tile([C, TF], f16, tag="act0")
                nc.scalar.activation(
                    out=a0[:], in_=tapwin(s, r0, T_ACT), func=AT.Copy,
                    scale=fT[:, T_ACT:T_ACT + 1],
                )
                # DVE stt merges psum + tap -> fp16 res (flat)
                rf = rts[s][:, r0 * W:(r0 + TROWS) * W]
                t = T_MERGE
                nc.vector.scalar_tensor_tensor(
                    out=rf, in0=tapwin(s, r0, t),
                    scalar=fT[:, t:t + 1], in1=pt[:],
                    op0=OP.mult, op1=OP.add,
                )
                # GPSIMD folds the ACT tap into res
                nc.gpsimd.tensor_add(rf, rf, a0[:])
                # subtract the wrapped horizontal-pad contributions
                rv = rf.rearrange("p (r w) -> p r w", w=W)
                nc.vector.scalar_tensor_tensor(
                    out=rv[:, :, 0:1],
                    in0=corr0s[s][:, r0:r0 + TROWS].rearrange(
                        "p (r w) -> p r w", w=1
                    ),
                    scalar=-1.0, in1=rv[:, :, 0:1],
                    op0=OP.mult, op1=OP.add,
                )
                nc.vector.scalar_tensor_tensor(
                    out=rv[:, :, W - 1:W],
                    in0=corr127s[s][:, r0:r0 + TROWS].rearrange(
                        "p (r w) -> p r w", w=1
                    ),
                    scalar=-1.0, in1=rv[:, :, W - 1:W],
                    op0=OP.mult, op1=OP.add,
                )
            def conv_tile_back(k):
                """Deferred res-chain tail: runs 2 tiles behind the front so
                the GPSIMD fold is never on the DVE serial path."""
                s, i = divmod(k, NT)
                r0 = i * TROWS
                fT = fTs[s]
                rf = rts[s][:, r0 * W:(r0 + TROWS) * W]
                dve_taps = ([] if k % 2 == 0 else [T_SHARED]) + [T_RMW]
                for t in dve_taps:
                    nc.vector.scalar_tensor_tensor(
                        out=rf, in0=tapwin(s, r0, t),
                        scalar=fT[:, t:t + 1], in1=rf,
                        op0=OP.mult, op1=OP.add,
                    )
                t = T_LAST
                nc.vector.scalar_tensor_tensor(
                    out=rf, in0=tapwin(s, r0, t),
                    scalar=fT[:, t:t + 1], in1=rf,
                    op0=OP.mult, op1=OP.add,
                    accum_out=sums[:, k:k + 1],
                )
                # ACT sumsq (Square + accum)
                sq = sqp.tile([C, TF], f16, tag="sq")
                nc.scalar.activation(
                    out=sq[:], in_=rf, func=AT.Square,
                    accum_out=sumsq[:, k:k + 1],
                )

            # schedule: prep plane 0, some plane-0 tiles, prep plane 1
            # (overlaps with remaining plane-0 conv), then the rest; the
            # res-chain tail trails the front by 2 tiles
            plane_prep_act(0)
            plane_prep_dve(0)
            conv_tile(0)
            plane_prep_act(1)
            for k in range(1, NT - 2):
                conv_tile(k)
                if k >= 2:
                    conv_tile_back(k - 2)
            plane_prep_dve(1)
            for k in range(NT - 2, NTILE):
                conv_tile(k)
                conv_tile_back(k - 2)
            conv_tile_back(NTILE - 2)
            conv_tile_back(NTILE - 1)

            # ---------------- sync-BN stats AllReduce
            arin = statp.tile([C, 2], f32, tag="arin")
            nc.vector.tensor_reduce(out=arin[:, 0:1], in_=sums[:], axis=AX.X, op=OP.add)
            nc.vector.tensor_reduce(out=arin[:, 1:2], in_=sumsq[:], axis=AX.X, op=OP.add)
            d_in = dram.tile([C, 2], f32, tag="d_in")
            d_out = dram.tile([C, 2], f32, tag="d_out")
            nc.sync.dma_start(out=d_in[:], in_=arin[:])
            nc.gpsimd.collective_compute(
                "AllReduce",
                OP.add,
                replica_groups=[list(range(n_cores))],
                ins=[d_in[:].opt()],
                outs=[d_out[:].opt()],
            )
            aro = statp.tile([C, 2], f32, tag="aro")
            nc.sync.dma_start(out=aro[:], in_=d_out[:])

            # ---------------- BN scale/shift (all [C,1], fp32)
            mean = statp.tile([C, 1], f32, tag="mean")
            nc.vector.tensor_scalar_mul(mean[:], aro[:, 0:1], 1.0 / ntot)
            ex2 = statp.tile([C, 1], f32, tag="ex2")
            nc.vector.tensor_scalar_mul(ex2[:], aro[:, 1:2], 1.0 / ntot)
            var = statp.tile([C, 1], f32, tag="var")
            nc.vector.tensor_mul(var[:], mean[:], mean[:])
            nc.vector.tensor_sub(var[:], ex2[:], var[:])
            veps = statp.tile([C, 1], f32, tag="veps")
            nc.vector.tensor_scalar_add(veps[:], var[:], BN_EPS)
            eps_t = statp.tile([C, 1], f32, tag="eps_t")
            nc.vector.memset(eps_t[:], BN_EPS)
            sd = statp.tile([C, 1], f32, tag="sd")
            nc.scalar.activation(out=sd[:], in_=var[:], func=AT.Sqrt, bias=eps_t[:])
            z = statp.tile([C, 1], f32, tag="z")
            nc.vector.reciprocal(z[:], sd[:])
            # one Newton step: z <- z * (1.5 - 0.5 * veps * z^2)
            nt = statp.tile([C, 1], f32, tag="nt")
            nc.vector.tensor_mul(nt[:], z[:], z[:])
            nc.vector.tensor_mul(nt[:], nt[:], veps[:])
            nc.vector.tensor_scalar(
                out=nt[:], in0=nt[:], scalar1=-0.5, scalar2=1.5,
                op0=OP.mult, op1=OP.add,
            )
            nc.vector.tensor_mul(z[:], z[:], nt[:])
            scale_t = statp.tile([C, 1], f32, tag="scale_t")
            nc.vector.tensor_mul(scale_t[:], gamma_s[:], z[:])
            shift_t = statp.tile([C, 1], f32, tag="shift_t")
            nc.vector.tensor_mul(shift_t[:], mean[:], scale_t[:])
            nc.vector.tensor_sub(shift_t[:], beta_s[:], shift_t[:])

            # ---------------- BN apply + ReLU (fp16) + writeback
            # small chunks alternating ACT/DVE so both engines overlap and
            # the first output DMA starts right after the AllReduce
            NCH = 8                      # chunks per plane
            CF = HW // NCH               # 2048
            for c in range(NCH):
                for s in range(BL):
                    cv = rts[s][:, c * CF:(c + 1) * CF]
                    if (s * NCH + c) % 2 == 0:
                        nc.scalar.activation(
                            out=cv, in_=cv, func=AT.Relu,
                            scale=scale_t[:], bias=shift_t[:],
                        )
                    else:
                        nc.vector.tensor_scalar(
                            out=cv, in0=cv,
                            scalar1=scale_t[:], scalar2=shift_t[:],
                            op0=OP.mult, op1=OP.add,
                        )
                        nc.vector.tensor_scalar_max(cv, cv, 0.0)
                    nc.sync.dma_start(
                        out=y_d[s, :, c * CF:(c + 1) * CF], in_=cv,
                    )

    nc.compile()
    return nc


_NC_CACHE = {}


def _get_nc(n_cores: int = N_CORES):
    if n_cores not in _NC_CACHE:
        _NC_CACHE[n_cores] = build_nc(n_cores)
    return _NC_CACHE[n_cores]


def make_in_maps(x: np.ndarray, gamma: np.ndarray, beta: np.ndarray,
                 n_cores: int = N_CORES):
    x_r = np.ascontiguousarray(
        np.asarray(x, dtype=np.float32).reshape(B, C, HW).astype(np.float16)
    )
    g = np.ascontiguousarray(np.asarray(gamma, dtype=np.float32).reshape(C, 1))
    b = np.ascontiguousarray(np.asarray(beta, dtype=np.float32).reshape(C, 1))
    ident = np.eye(C, dtype=np.float16)
    crecip = _counts_recip()
    maps = []
    for core in range(n_cores):
        maps.append({
            "x": x_r[core * BL:(core + 1) * BL],
            "gamma": g,
            "beta": b,
            "ident": ident,
            "crecip": crecip,
        })
    return maps


def kernel(x, gamma, beta):
    from concourse import bass_utils

    nc = _get_nc(N_CORES)
    in_maps = make_in_maps(x, gamma, beta, N_CORES)
    res = bass_utils.run_bass_kernel_spmd(nc, in_maps, core_ids=list(range(N_CORES)))
    y = np.concatenate([res.results[c]["y"] for c in range(N_CORES)], axis=0)
    return y.reshape(B, C, H, W).astype(np.float32)


# revision 30
# speedup vs baseline: 1.1229x; 1.1229x over previous
"""DCM (dynamic conv module) Trainium2 kernel, v3.

Reference computation (per sample b, channel c):
  f[b,c,3,3]  = adaptive_avg_pool2d(x[b,c], 3)        # dynamic depthwise filter
  out[b,c]    = depthwise_conv3x3(x[b,c], f[b,c])     # zero padding 1
  y           = relu(batchnorm_train(out, gamma, beta))  # batch stats over (B,H,W)

Sharding: data-parallel over batch B=16 across 8 cores (2 samples/core).
Sync-BN via a [C,2] AllReduce of per-channel (sum, sumsq).

v3 design:
  - x shipped fp16 (host cast), resident in SBUF once per plane in a PACKED
    layout [C, 1 + 130*128 + 1]: one zero row above and below the 128 x rows
    plus a single zero element at each end. Rows are contiguous (stride W),
    so every tap window is a FLAT [C, 2048] slice: fast contiguous DMA in,
    fast PE moving operands, clean 2D DVE/ACT access patterns.
  - Horizontal zero-padding: flat dj=+-1 windows wrap across row ends; the
    wrapped contributions are removed by per-plane correction columns
    (corr0/corr127 [C,128], 3 small stt each) subtracted from the two edge
    columns of each result tile (2 small strided stt per tile).
  - The 9 taps are spread across engines (measured rates: PE ~0.77ns/col,
    ACT ~1.2GHz, DVE ~1cyc/elem, no DVE fast mode for accumulating ops):
      ACT 1 tap  (0,0): activation Copy w/ per-channel scale writes PSUM
      PE  5 taps: fp16 diag matmuls accumulate on top (start=False)
      DVE 3 taps: stt merge psum->res (fp16), then 2 stt RMW on res;
                  the last one emits the per-channel sum via accum_out
  - Pooling on ACT as 9 rectangle sums per plane (Copy + accum_out);
    sumsq on ACT (Square + accum_out). Same activation table -> no reloads.
  - BN apply on DVE/ACT in fp16, output DMA'd fp16; host casts to f32.
"""

import numpy as np

# ---------------------------------------------------------------- constants
B, C, H, W = 16, 128, 128, 128
N_CORES = 8
BL = B // N_CORES          # samples per core
HW = H * W
FS = 3
BN_EPS = 1e-5

XF = 1 + (H + 3) * W       # packed plane: 1 front pad, zero row, 128 x rows,
                           # zero row, then a zero row of slack for views
XOFF = 1 + W               # flat offset of x row 0 inside the plane tile

TROWS = 16                 # output rows per psum tile
NT = H // TROWS            # tiles per plane (8)
TF = TROWS * W             # 2048 free elems per psum tile
NTILE = NT * BL            # psum tiles per core (16)
MM_N = 512                 # max moving free per matmul (psum bank limit)

# adaptive_avg_pool2d(3) bin boundaries (PyTorch convention)
SH = [(i * H) // FS for i in range(FS)]
EH = [-((-(i + 1) * H) // FS) for i in range(FS)]
SW = [(i * W) // FS for i in range(FS)]
EW = [-((-(i + 1) * W) // FS) for i in range(FS)]

# taps t = 3*(di+1)+(dj+1), (di,dj) in {-1,0,1}^2
TAPS = [(di, dj) for di in (-1, 0, 1) for dj in (-1, 0, 1)]
T_ACT = 4                  # (0,0)  ACT tap (SBUF fp16, folded by GPSIMD)
T_PE = [0, 1, 2, 6, 7]     # (-1,*), (1,-1), (1,0): PE diag matmuls
T_MERGE = 5                # (0,1)  DVE stt: res = x*f + psum (fp16 out)
T_RMW = 3                  # (0,-1) DVE stt RMW on res
T_LAST = 8                 # (1,1)  DVE stt RMW on res, accum_out -> sums
DJM = [0, 3, 6]            # taps with dj=-1 (wrap at col 0)
DJP = [2, 5, 8]            # taps with dj=+1 (wrap at col W-1)


def _counts_recip():
    cr = np.empty((C, FS * FS), dtype=np.float32)
    for i in range(FS):
        for j in range(FS):
            cr[:, 3 * i + j] = 1.0 / float((EH[i] - SH[i]) * (EW[j] - SW[j]))
    return cr


def build_nc(n_cores: int = N_CORES):
    """Build + compile the per-core Bass program (identical on all cores)."""
    import concourse.bacc as bacc
    import concourse.tile as tile
    from concourse import mybir

    f32 = mybir.dt.float32
    f16 = mybir.dt.float16
    AT = mybir.ActivationFunctionType
    OP = mybir.AluOpType
    AX = mybir.AxisListType

    ntot = float(n_cores * BL * HW)   # BN element count per channel

    nc = bacc.Bacc(
        "TRN2",
        target_bir_lowering=False,
        debug=False,
        num_devices=n_cores,
    )

    x_d = nc.dram_tensor("x", [BL, C, HW], f16, kind="ExternalInput").ap()
    gamma_d = nc.dram_tensor("gamma", [C, 1], f32, kind="ExternalInput").ap()
    beta_d = nc.dram_tensor("beta", [C, 1], f32, kind="ExternalInput").ap()
    ident_d = nc.dram_tensor("ident", [C, C], f16, kind="ExternalInput").ap()
    crecip_d = nc.dram_tensor("crecip", [C, FS * FS], f32, kind="ExternalInput").ap()
    y_d = nc.dram_tensor("y", [BL, C, HW], f16, kind="ExternalOutput").ap()

    with tile.TileContext(nc) as tc:
        with (
            tc.tile_pool(name="singles", bufs=1) as singles,
            tc.tile_pool(name="xres", bufs=BL) as xresp,
            tc.tile_pool(name="res", bufs=BL) as resp,
            tc.tile_pool(name="psum", bufs=2, space="PSUM") as psum,
            tc.tile_pool(name="scr", bufs=2) as scrp,
            tc.tile_pool(name="sq", bufs=2) as sqp,
            tc.tile_pool(name="fpool", bufs=2) as fpool,
            tc.tile_pool(name="diagp", bufs=2 * len(T_PE)) as diagp,
            tc.tile_pool(name="corrp", bufs=2 * 2) as corrp,
            tc.tile_pool(name="statp", bufs=1) as statp,
            tc.tile_pool(name="dram", bufs=1, space="DRAM") as dram,
        ):
            # ---- constants
            gamma_s = singles.tile([C, 1], f32, tag="gamma")
            nc.sync.dma_start(out=gamma_s[:], in_=gamma_d[:, :])
            beta_s = singles.tile([C, 1], f32, tag="beta")
            nc.sync.dma_start(out=beta_s[:], in_=beta_d[:, :])
            ident_s = singles.tile([C, C], f16, tag="ident")
            nc.sync.dma_start(out=ident_s[:], in_=ident_d[:, :])
            crecip_s = singles.tile([C, FS * FS], f32, tag="crecip")
            nc.sync.dma_start(out=crecip_s[:], in_=crecip_d[:, :])

            sums = statp.tile([C, NTILE], f32, tag="sums")
            sumsq = statp.tile([C, NTILE], f32, tag="sumsq")

            # Dummy warm-up AllReduce issued at kernel start: absorbs the
            # one-time ncfw ramp so the real stats AllReduce is cheaper.
            warm = statp.tile([C, 2], f32, tag="warm")
            nc.gpsimd.memset(warm[:], 0.0)
            dw_in = dram.tile([C, 2], f32, tag="dw_in")
            dw_out = dram.tile([C, 2], f32, tag="dw_out")
            nc.sync.dma_start(out=dw_in[:], in_=warm[:])
            for _ in range(2):
                nc.gpsimd.collective_compute(
                    "AllReduce",
                    OP.add,
                    replica_groups=[list(range(n_cores))],
                    ins=[dw_in[:].opt()],
                    outs=[dw_out[:].opt()],
                )

            # ---- resident x planes (packed, padded rows) + result planes
            xts, rts = [], []
            for s in range(BL):
                xt = xresp.tile([C, XF], f16, tag="xres")
                nc.vector.memset(xt[:, 0:XOFF], 0.0)            # front pad + row -1
                nc.vector.memset(xt[:, XOFF + HW:XF], 0.0)      # rows 128..129+
                xts.append(xt)
                rt = resp.tile([C, HW], f16, tag="res")
                rts.append(rt)

            # chunked contiguous plane loads; issue across three engine
            # queues in parallel (each dma_start costs ~0.7us of issue time
            # on its queue, which serializes the transfers if queued on one)
            NLCH = 8
            LCF = HW // NLCH
            for s in range(BL):
                for ci in range(NLCH):
                    nc.sync.dma_start(
                        out=xts[s][:, XOFF + ci * LCF:XOFF + (ci + 1) * LCF],
                        in_=x_d[s, :, ci * LCF:(ci + 1) * LCF],
                    )

            fTs, diags, corr0s, corr127s, rects_ = {}, {}, {}, {}, {}

            def plane_prep_act(s):
                """Pool rectangle sums on ACT (emit early)."""
                xt = xts[s]
                xv = xt[:, XOFF:XOFF + HW].rearrange("p (r w) -> p r w", w=W)
                rect = fpool.tile([C, FS * FS], f32, tag="rect")
                scr = scrp.tile([C, 44 * 44], f16, tag="scr")
                for i in range(FS):
                    bh = EH[i] - SH[i]
                    for j in range(FS):
                        bw = EW[j] - SW[j]
                        sv = scr[:, 0:bh * bw].rearrange("p (r w) -> p r w", w=bw)
                        nc.scalar.activation(
                            out=sv,
                            in_=xv[:, SH[i]:EH[i], SW[j]:EW[j]],
                            func=AT.Copy,
                            accum_out=rect[:, 3 * i + j:3 * i + j + 1],
                        )
                rects_[s] = rect

            def plane_prep_dve(s):
                """f, diag weights, wrap-correction cols (DVE, emit late
                enough that the pool rects have landed)."""
                xt = xts[s]
                fT = fpool.tile([C, FS * FS], f32, tag="fT")
                nc.vector.tensor_mul(fT[:], rects_[s][:], crecip_s[:])
                fTs[s] = fT
                dgs = {}
                for t in T_PE:
                    dg = diagp.tile([C, C], f16, tag="diag")
                    nc.vector.tensor_scalar_mul(dg[:], ident_s[:], fT[:, t:t + 1])
                    dgs[t] = dg
                diags[s] = dgs
                # wrap-correction columns:
                # corr0[r]   = sum_di f(di,-1) * xpad[row r+di,   col 127]
                # corr127[r] = sum_di f(di,+1) * xpad[row r+di+2, col 0]
                c0 = corrp.tile([C, H], f32, tag="corr0")
                c1 = corrp.tile([C, H], f32, tag="corr127")
                for n, t in enumerate(DJM):
                    di = TAPS[t][0]
                    a = (di + 1) * W
                    v = xt[:, a:a + HW].rearrange(
                        "p (r w) -> p r w", w=W
                    )[:, :, 0:1]
                    if n == 0:
                        nc.vector.tensor_scalar_mul(
                            c0[:].rearrange("p (r w) -> p r w", w=1),
                            v, fT[:, t:t + 1],
                        )
                    else:
                        nc.vector.scalar_tensor_tensor(
                            out=c0[:].rearrange("p (r w) -> p r w", w=1),
                            in0=v, scalar=fT[:, t:t + 1],
                            in1=c0[:].rearrange("p (r w) -> p r w", w=1),
                            op0=OP.mult, op1=OP.add,
                        )
                for n, t in enumerate(DJP):
                    di = TAPS[t][0]
                    a = 1 + (di + 2) * W
                    v = xt[:, a:a + HW].rearrange(
                        "p (r w) -> p r w", w=W
                    )[:, :, 0:1]
                    if n == 0:
                        nc.vector.tensor_scalar_mul(
                            c1[:].rearrange("p (r w) -> p r w", w=1),
                            v, fT[:, t:t + 1],
                        )
                    else:
                        nc.vector.scalar_tensor_tensor(
                            out=c1[:].rearrange("p (r w) -> p r w", w=1),
                            in0=v, scalar=fT[:, t:t + 1],
                            in1=c1[:].rearrange("p (r w) -> p r w", w=1),
                            op0=OP.mult, op1=OP.add,
                        )
                corr0s[s], corr127s[s] = c0, c1

            def tapwin(s, r0, t, n=TF):
                """Flat [C, n] window of x for tap t covering out rows r0.."""
                di, dj = TAPS[t]
                ofs = 1 + (r0 + di + 1) * W + dj
                return xts[s][:, ofs:ofs + n]

            def conv_tile(k):
                s, i = divmod(k, NT)
                r0 = i * TROWS
                fT = fTs[s]
                pt = psum.tile([C, TF], f32, tag="pt")
                # PE diag-matmul taps accumulate in psum
                for n, t in enumerate(T_PE):
                    last = n == len(T_PE) - 1
                    for h in range(TF // MM_N):
                        nc.tensor.matmul(
                            pt[:, h * MM_N:(h + 1) * MM_N],
                            diags[s][t][:],
                            tapwin(s, r0, t)[:, h * MM_N:(h + 1) * MM_N],
                            start=n == 0,
                            stop=last,
                        )
                # ACT computes tap T_ACT into an SBUF fp16 tile (folded in
                # below by a GPSIMD tensor add)
                a0 = sqp.tile([C, TF], f16, tag="act0")
                nc.scalar.activation(
                    out=a0[:], in_=tapwin(s, r0, T_ACT), func=AT.Copy,
                    scale=fT[:, T_ACT:T_ACT + 1],
                )
                # DVE stt merges psum + tap -> fp16 res (flat)
                rf = rts[s][:, r0 * W:(r0 + TROWS) * W]
                t = T_MERGE
                nc.vector.scalar_tensor_tensor(
                    out=rf, in0=tapwin(s, r0, t),
                    scalar=fT[:, t:t + 1], in1=pt[:],
                    op0=OP.mult, op1=OP.add,
                )
                # GPSIMD folds the ACT tap into res
                nc.gpsimd.tensor_add(rf, rf, a0[:])
                # subtract the wrapped horizontal-pad contributions
                rv = rf.rearrange("p (r w) -> p r w", w=W)
                nc.vector.scalar_tensor_tensor(
                    out=rv[:, :, 0:1],
                    in0=corr0s[s][:, r0:r0 + TROWS].rearrange(
                        "p (r w) -> p r w", w=1
                    ),
                    scalar=-1.0, in1=rv[:, :, 0:1],
                    op0=OP.mult, op1=OP.add,
                )
                nc.vector.scalar_tensor_tensor(
                    out=rv[:, :, W - 1:W],
                    in0=corr127s[s][:, r0:r0 + TROWS].rearrange(
                        "p (r w) -> p r w", w=1
                    ),
                    scalar=-1.0, in1=rv[:, :, W - 1:W],
                    op0=OP.mult, op1=OP.add,
                )
            def conv_tile_back(k):
                """Deferred res-chain tail: runs 2 tiles behind the front so
                the GPSIMD fold is never on the DVE serial path."""
                s, i = divmod(k, NT)
                r0 = i * TROWS
                fT = fTs[s]
                rf = rts[s][:, r0 * W:(r0 + TROWS) * W]
                t = T_RMW
                nc.vector.scalar_tensor_tensor(
                    out=rf, in0=tapwin(s, r0, t),
                    scalar=fT[:, t:t + 1], in1=rf,
                    op0=OP.mult, op1=OP.add,
                )
                t = T_LAST
                nc.vector.scalar_tensor_tensor(
                    out=rf, in0=tapwin(s, r0, t),
                    scalar=fT[:, t:t + 1], in1=rf,
                    op0=OP.mult, op1=OP.add,
                    accum_out=sums[:, k:k + 1],
                )
                # ACT sumsq (Square + accum)
                sq = sqp.tile([C, TF], f16, tag="sq")
                nc.scalar.activation(
                    out=sq[:], in_=rf, func=AT.Square,
                    accum_out=sumsq[:, k:k + 1],
                )

            # schedule: prep plane 0, some plane-0 tiles, prep plane 1
            # (overlaps with remaining plane-0 conv), then the rest; the
            # res-chain tail trails the front by 2 tiles
            plane_prep_act(0)
            plane_prep_dve(0)
            conv_tile(0)
            plane_prep_act(1)
            for k in range(1, NTILE):
                conv_tile(k)
                if k >= 2:
                    conv_tile_back(k - 2)
                if k == 3:
                    plane_prep_dve(1)
            conv_tile_back(NTILE - 2)
            conv_tile_back(NTILE - 1)

            # ---------------- sync-BN stats AllReduce
            arin = statp.tile([C, 2], f32, tag="arin")
            nc.vector.tensor_reduce(out=arin[:, 0:1], in_=sums[:], axis=AX.X, op=OP.add)
            nc.vector.tensor_reduce(out=arin[:, 1:2], in_=sumsq[:], axis=AX.X, op=OP.add)
            d_in = dram.tile([C, 2], f32, tag="d_in")
            d_out = dram.tile([C, 2], f32, tag="d_out")
            nc.sync.dma_start(out=d_in[:], in_=arin[:])
            nc.gpsimd.collective_compute(
                "AllReduce",
                OP.add,
                replica_groups=[list(range(n_cores))],
                ins=[d_in[:].opt()],
                outs=[d_out[:].opt()],
            )
            aro = statp.tile([C, 2], f32, tag="aro")
            nc.sync.dma_start(out=aro[:], in_=d_out[:])

            # ---------------- BN scale/shift (all [C,1], fp32)
            mean = statp.tile([C, 1], f32, tag="mean")
            nc.vector.tensor_scalar_mul(mean[:], aro[:, 0:1], 1.0 / ntot)
            ex2 = statp.tile([C, 1], f32, tag="ex2")
            nc.vector.tensor_scalar_mul(ex2[:], aro[:, 1:2], 1.0 / ntot)
            var = statp.tile([C, 1], f32, tag="var")
            nc.vector.tensor_mul(var[:], mean[:], mean[:])
            nc.vector.tensor_sub(var[:], ex2[:], var[:])
            veps = statp.tile([C, 1], f32, tag="veps")
            nc.vector.tensor_scalar_add(veps[:], var[:], BN_EPS)
            eps_t = statp.tile([C, 1], f32, tag="eps_t")
            nc.vector.memset(eps_t[:], BN_EPS)
            sd = statp.tile([C, 1], f32, tag="sd")
            nc.scalar.activation(out=sd[:], in_=var[:], func=AT.Sqrt, bias=eps_t[:])
            z = statp.tile([C, 1], f32, tag="z")
            nc.vector.reciprocal(z[:], sd[:])
            # one Newton step: z <- z * (1.5 - 0.5 * veps * z^2)
            nt = statp.tile([C, 1], f32, tag="nt")
            nc.vector.tensor_mul(nt[:], z[:], z[:])
            nc.vector.tensor_mul(nt[:], nt[:], veps[:])
            nc.vector.tensor_scalar(
                out=nt[:], in0=nt[:], scalar1=-0.5, scalar2=1.5,
                op0=OP.mult, op1=OP.add,
            )
            nc.vector.tensor_mul(z[:], z[:], nt[:])
            scale_t = statp.tile([C, 1], f32, tag="scale_t")
            nc.vector.tensor_mul(scale_t[:], gamma_s[:], z[:])
            shift_t = statp.tile([C, 1], f32, tag="shift_t")
            nc.vector.tensor_mul(shift_t[:], mean[:], scale_t[:])
            nc.vector.tensor_sub(shift_t[:], beta_s[:], shift_t[:])

            # ---------------- BN apply + ReLU (fp16) + writeback
            # small chunks alternating ACT/DVE so both engines overlap and
            # the first output DMA starts right after the AllReduce
            NCH = 8                      # chunks per plane
            CF = HW // NCH               # 2048
            for c in range(NCH):
                for s in range(BL):
                    cv = rts[s][:, c * CF:(c + 1) * CF]
                    if (s * NCH + c) % 2 == 0:
                        nc.scalar.activation(
                            out=cv, in_=cv, func=AT.Relu,
                            scale=scale_t[:], bias=shift_t[:],
                        )
                    else:
                        nc.vector.tensor_scalar(
                            out=cv, in0=cv,
                            scalar1=scale_t[:], scalar2=shift_t[:],
                            op0=OP.mult, op1=OP.add,
                        )
                        nc.vector.tensor_scalar_max(cv, cv, 0.0)
                    nc.sync.dma_start(
                        out=y_d[s, :, c * CF:(c + 1) * CF], in_=cv,
                    )

    nc.compile()
    return nc


_NC_CACHE = {}


def _get_nc(n_cores: int = N_CORES):
    if n_cores not in _NC_CACHE:
        _NC_CACHE[n_cores] = build_nc(n_cores)
    return _NC_CACHE[n_cores]


def make_in_maps(x: np.ndarray, gamma: np.ndarray, beta: np.ndarray,
                 n_cores: int = N_CORES):
    x_r = np.ascontiguousarray(
        np.asarray(x, dtype=np.float32).reshape(B, C, HW).astype(np.float16)
    )
    g = np.ascontiguousarray(np.asarray(gamma, dtype=np.float32).reshape(C, 1))
    b = np.ascontiguousarray(np.asarray(beta, dtype=np.float32).reshape(C, 1))
    ident = np.eye(C, dtype=np.float16)
    crecip = _counts_recip()
    maps = []
    for core in range(n_cores):
        maps.append({
            "x": x_r[core * BL:(core + 1) * BL],
            "gamma": g,
            "beta": b,
            "ident": ident,
            "crecip": crecip,
        })
    return maps


def kernel(x, gamma, beta):
    from concourse import bass_utils

    nc = _get_nc(N_CORES)
    in_maps = make_in_maps(x, gamma, beta, N_CORES)
    res = bass_utils.run_bass_kernel_spmd(nc, in_maps, core_ids=list(range(N_CORES)))
    y = np.concatenate([res.results[c]["y"] for c in range(N_CORES)], axis=0)
    return y.reshape(B, C, H, W).astype(np.float32)
